# revision 8
# baseline (speedup 1.0000x reference)
"""Bahdanau additive attention for Trainium2, data-parallel over batch on 8 cores.

Per core (one batch element):
  mp[k,s] = (Wa_m.T @ memory.T)      via PE (memory transposed on-chip)
  dp[k,t] = (Wa_d.T @ dec.T)
  for each t:  e[t,s] = Va . tanh(mp[:,s] + dp[:,t])
    - adds on DVE (tensor_scalar, per-partition scalar dp[:,t])
    - tanh on ACT (bf16 out)
    - Va-contraction on PE as m=1 matvecs into 32-aligned PSUM rows
  softmax over s without max-subtraction (|e| <= sum|Va| ~ 18, exp safe in fp32),
  masked by multiplying exp(e) with the mask, then context = softmax @ memory.

All pools are flat (no scoped address reuse): PSUM tags fit the 8 banks exactly.
"""
import os
import numpy as np

B, SRC, TGT, ENC, DEC = 8, 512, 128, 512, 512
N_CORES = 8
SN, KN, EN = SRC // 128, DEC // 128, ENC // 128
TG = 8            # t-groups
TPG = TGT // TG   # 16 t per group
RPG = TPG // 4    # 4 rounds per group

TRACE = bool(int(os.environ.get("KERNEL_TRACE", "0")))

_compiled = None


def _build():
    import concourse.bacc as bacc
    import concourse.bass as bass
    import concourse.tile as tile
    from concourse import mybir
    from concourse.masks import make_identity

    f32 = mybir.dt.float32
    bf16 = mybir.dt.bfloat16
    u8 = mybir.dt.uint8
    AF = mybir.ActivationFunctionType

    nc = bacc.Bacc()
    mem_d = nc.dram_tensor("mem", [SRC, ENC], f32, kind="ExternalInput")
    dec_d = nc.dram_tensor("dec", [TGT, DEC], f32, kind="ExternalInput")
    mask_d = nc.dram_tensor("mask", [SRC], u8, kind="ExternalInput")
    wa_d = nc.dram_tensor("Wa", [ENC + DEC, DEC], f32, kind="ExternalInput")
    va_d = nc.dram_tensor("Va", [DEC], f32, kind="ExternalInput")
    out_d = nc.dram_tensor("out", [TGT, ENC], f32, kind="ExternalOutput")

    with tile.TileContext(nc) as tc:
        with tc.tile_pool(name="const", bufs=1) as cpool, \
             tc.tile_pool(name="prep", bufs=1) as pp, \
             tc.tile_pool(name="xp", bufs=2) as xp, \
             tc.tile_pool(name="thp", bufs=2) as thp, \
             tc.tile_pool(name="scrp", bufs=3) as scrp, \
             tc.tile_pool(name="post", bufs=1) as post, \
             tc.tile_pool(name="ps", bufs=1, space="PSUM") as ps:
            # ---- statics ----
            va_col = cpool.tile([128, KN], f32)
            nc.sync.dma_start(out=va_col, in_=va_d.ap().rearrange("(a b) -> b a", a=KN))
            va_bf = cpool.tile([128, KN], bf16)
            nc.vector.tensor_copy(va_bf, va_col)

            mask_u8 = cpool.tile([128, SRC], u8)
            mask_bcast = bass.AP(tensor=mask_d, offset=0, ap=[[0, 128], [1, SRC]])
            nc.sync.dma_start(out=mask_u8, in_=mask_bcast)
            mask_bf = cpool.tile([128, SRC], bf16)
            nc.vector.tensor_copy(mask_bf, mask_u8)

            mem_bf = [cpool.tile([128, ENC], bf16, tag=f"membf{i}", name=f"membf{i}") for i in range(SN)]
            mpT = [cpool.tile([128, SRC], f32, tag=f"mpT{i}", name=f"mpT{i}") for i in range(KN)]
            dpT = [cpool.tile([128, TGT], f32, tag=f"dpT{i}", name=f"dpT{i}") for i in range(KN)]
            e_sb = cpool.tile([128, SRC], f32)

            zero_st = cpool.tile([128, 128], bf16)
            nc.vector.memset(zero_st, 0.0)

            ident = cpool.tile([128, 128], f32)
            make_identity(nc, ident)
            ident_bf = cpool.tile([128, 128], bf16)
            nc.vector.tensor_copy(ident_bf, ident)

            # ---- prep: loads, transposes, projections ----
            mem_sb = [pp.tile([128, ENC], f32, tag=f"mem{i}", name=f"mem{i}") for i in range(SN)]
            for i in range(SN):
                nc.sync.dma_start(out=mem_sb[i], in_=mem_d.ap()[i * 128:(i + 1) * 128, :])
                nc.vector.tensor_copy(mem_bf[i], mem_sb[i])
            dec_sb = pp.tile([128, DEC], f32)
            nc.sync.dma_start(out=dec_sb, in_=dec_d.ap())
            wad = [pp.tile([128, DEC], f32, tag=f"wad{i}", name=f"wad{i}") for i in range(EN)]
            wam = [pp.tile([128, DEC], f32, tag=f"wam{i}", name=f"wam{i}") for i in range(EN)]
            for i in range(EN):
                nc.sync.dma_start(out=wad[i], in_=wa_d.ap()[i * 128:(i + 1) * 128, :])
                nc.sync.dma_start(out=wam[i], in_=wa_d.ap()[ENC + i * 128:ENC + (i + 1) * 128, :])

            memT = [pp.tile([128, SRC], f32, tag=f"memT{i}", name=f"memT{i}") for i in range(EN)]
            decT = [pp.tile([128, TGT], f32, tag=f"decT{i}", name=f"decT{i}") for i in range(EN)]
            for en in range(EN):
                for sn in range(SN):
                    ptr = ps.tile([128, 128], f32, tag="tr", bufs=2)
                    nc.tensor.transpose(ptr, mem_sb[sn][:, en * 128:(en + 1) * 128], ident)
                    nc.vector.tensor_copy(memT[en][:, sn * 128:(sn + 1) * 128], ptr)
                ptr2 = ps.tile([128, 128], f32, tag="tr", bufs=2)
                nc.tensor.transpose(ptr2, dec_sb[:, en * 128:(en + 1) * 128], ident)
                nc.vector.tensor_copy(decT[en], ptr2)

            for kn in range(KN):
                pmp = ps.tile([128, SRC], f32, tag="mp")
                for en in range(EN):
                    nc.tensor.matmul(pmp, lhsT=wam[en][:, kn * 128:(kn + 1) * 128],
                                     rhs=memT[en], start=(en == 0), stop=(en == EN - 1))
                nc.vector.tensor_copy(mpT[kn], pmp)
                pdp = ps.tile([128, TGT], f32, tag="dp")
                for en in range(EN):
                    nc.tensor.matmul(pdp, lhsT=wad[en][:, kn * 128:(kn + 1) * 128],
                                     rhs=decT[en], start=(en == 0), stop=(en == EN - 1))
                nc.vector.tensor_copy(dpT[kn], pdp)

            # ---- main loop ----
            for g in range(TG):
                prnd = [ps.tile([128, SRC], f32, tag=f"rnd{j}", name=f"rnd_g{g}_{j}") for j in range(RPG)]
                for j in range(RPG):
                    # zero-fill all 128 partitions so the later full-tile copy
                    # never reads uninitialized PSUM (only 4 rows get matvecs)
                    nc.tensor.matmul(prnd[j], lhsT=zero_st, rhs=mem_bf[0],
                                     start=True, stop=False)
                for kn in range(KN):
                    x = xp.tile([128, TPG * SRC], f32)
                    for lt in range(TPG):
                        t = g * TPG + lt
                        nc.vector.tensor_scalar_add(
                            x[:, lt * SRC:(lt + 1) * SRC], mpT[kn], dpT[kn][:, t:t + 1])
                    th = thp.tile([128, TPG * SRC], bf16)
                    nc.scalar.activation(out=th, in_=x, func=AF.Tanh)
                    for j in range(RPG):
                        for i in range(4):
                            lt = 4 * j + i
                            nc.tensor.matmul(
                                prnd[j][32 * i:32 * i + 1, :],
                                lhsT=va_bf[:, kn:kn + 1],
                                rhs=th[:, lt * SRC:(lt + 1) * SRC],
                                start=False, stop=False,
                                tile_position=(0, 32 * i))
                for j in range(RPG):
                    # close the accumulation group on every element
                    nc.tensor.matmul(prnd[j], lhsT=zero_st, rhs=mem_bf[0],
                                     start=False, stop=True)
                    scr = scrp.tile([128, SRC], f32)
                    nc.vector.tensor_copy(scr, prnd[j])
                    t0 = g * TPG + 4 * j
                    nc.sync.dma_start(out=e_sb[t0:t0 + 4, :], in_=scr[0:128:32, :])

            # ---- softmax + context ----
            s_bf = post.tile([128, SRC], bf16)
            nc.scalar.activation(out=s_bf, in_=e_sb, func=AF.Exp)
            nc.vector.tensor_mul(s_bf, s_bf, mask_bf)
            z = post.tile([128, 2], f32)
            nc.vector.reduce_sum(z[:, 0:1], s_bf, axis=mybir.AxisListType.X)
            nc.vector.reciprocal(z[:, 1:2], z[:, 0:1])

            sT = [post.tile([128, TGT], bf16, tag=f"sT{i}", name=f"sT{i}") for i in range(SN)]
            for sn in range(SN):
                ptr3 = ps.tile([128, 128], bf16, tag="tr", bufs=2)
                nc.tensor.transpose(ptr3, s_bf[:, sn * 128:(sn + 1) * 128], ident_bf)
                nc.vector.tensor_copy(sT[sn], ptr3)

            pctx = ps.tile([128, ENC], f32, tag="mp", name="pctx")
            for sn in range(SN):
                nc.tensor.matmul(pctx, lhsT=sT[sn], rhs=mem_bf[sn],
                                 start=(sn == 0), stop=(sn == SN - 1))
            ctx = post.tile([128, ENC], f32)
            nc.vector.tensor_scalar_mul(ctx, pctx, z[:, 1:2])
            nc.sync.dma_start(out=out_d.ap(), in_=ctx)

    nc.compile()
    return nc


def kernel(memory, decoder_state, mask, Wa, Va):
    from concourse.bass_utils import run_bass_kernel_spmd

    global _compiled
    if _compiled is None:
        _compiled = _build()
    nc = _compiled

    memory = np.ascontiguousarray(np.asarray(memory, dtype=np.float32))
    decoder_state = np.ascontiguousarray(np.asarray(decoder_state, dtype=np.float32))
    mask_u8 = np.ascontiguousarray(np.asarray(mask).astype(np.uint8))
    Wa = np.ascontiguousarray(np.asarray(Wa, dtype=np.float32))
    Va = np.ascontiguousarray(np.asarray(Va, dtype=np.float32))

    in_maps = [
        {"mem": memory[i], "dec": decoder_state[i], "mask": mask_u8[i], "Wa": Wa, "Va": Va}
        for i in range(N_CORES)
    ]
    res = run_bass_kernel_spmd(nc, in_maps, core_ids=list(range(N_CORES)), trace=TRACE)
    if TRACE and res.exec_time_ns is not None:
        kernel.last_exec_time_ns = res.exec_time_ns
        kernel.last_mean_exec_time_ns = res.mean_exec_time_ns
    out = np.stack([res.results[i]["out"] for i in range(N_CORES)], axis=0)
    return out.astype(np.float32)


kernel.last_exec_time_ns = None
kernel.last_mean_exec_time_ns = None
